# revision 15
# baseline (speedup 1.0000x reference)
"""Multi-head attention (B=4, S=1024, D=1024, H=16) on 8 Trainium2 NeuronCores.

v2 — head-parallel sharding: core c handles batch b = c//2 and head-group
hh = c%2 (heads hh*8 .. hh*8+7, i.e. feature range hh*512 .. hh*512+512).
Each core computes:
  - Q/K/V projections restricted to its 512 features over the full 1024
    tokens of its batch (no duplicated projection work anywhere),
  - attention for its 8 heads (4 head-pairs packed 2-per-128-partitions),
  - a PARTIAL output projection (contraction over its 512 features only).
The two cores of a batch return fp32 partial outputs; the host sums them
during unshard (a partial-sum gather).  The V bias is folded into an
effective output bias on the host (bo_eff = bo + Wo @ bv, applied by the
hh=0 core only), so the V projection is a pure matmul.

On-chip layout (per core):
  - projections keep feature dims on partitions, token dims on the free
    axis; V projection produces [token, feature] tiles (vsb) with an
    appended ones-column so the AV matmul's 65th output row is the
    softmax denominator;
  - scores come out as S^T [k, q] (k on partitions), softmax is
    exp(score/8)*mask with no max subtraction (scores are O(1) and
    exp never overflows; masked weights are exactly 0);
  - inputs arrive as per-feature-tile DMA chunks in priority order so
    the first matmul starts ~1.5us into the kernel.

PSUM budget: score tiles [128,2,512] double-buffered (4 banks) + two AV
accumulators [65,1024] (4 banks) = 8 banks.
"""

import numpy as np
import ml_dtypes

import concourse.bass as bass
import concourse.tile as tile
from concourse import bacc, mybir
from concourse import bass_utils

B, S, D, H, DK = 4, 1024, 1024, 16, 64
NT = 8           # input-feature tiles (128 each) of D
NH = 8           # heads per core
NP = 4           # head pairs per core
FH = 512         # features per core (NH * DK)
NKT = 8          # k-token tiles
NOT = 8          # output feature tiles of D
NCORES = 8
DELAY = 2        # AV matmuls trail the exp stream by this many units
BF16 = mybir.dt.bfloat16
F32 = mybir.dt.float32
Exp = mybir.ActivationFunctionType.Exp
Identity = mybir.ActivationFunctionType.Identity
Mult = mybir.AluOpType.mult

_COMPILED = None
TRACE = False
TRACE_CORES = [0]
LAST_RESULT = None
BC64 = False


def _emit(nc, tc):
    dram = {n: nc.dram_tensor(n, shp, dt, kind="ExternalInput") for n, shp, dt in [
        ("qT", (NT, 128, S), BF16),
        ("kT", (NT, 128, S), BF16),
        ("vT", (NT, 128, S), BF16),
        ("mT", (NKT, 128, S), BF16),
        ("wq", (NT, 128, FH), BF16),
        ("wk", (NT, 128, FH), BF16),
        ("wv", (NT, 128, FH), BF16),
        ("wo", (NP, 128, D), BF16),
        ("bqc", (128, NP), F32),
        ("bkc", (128, NP), F32),
        ("boc", (128, NOT), F32),
    ]}
    outT = nc.dram_tensor("outT", (NOT, 128, S), BF16, kind="ExternalOutput")

    import contextlib
    stack = contextlib.ExitStack()
    with stack:
        wpool = stack.enter_context(tc.tile_pool(name="wpool", bufs=1))
        inpool = stack.enter_context(tc.tile_pool(name="inpool", bufs=1))
        acts = stack.enter_context(tc.tile_pool(name="acts", bufs=1))
        xp = stack.enter_context(tc.tile_pool(name="xp", bufs=1))
        small = stack.enter_context(tc.tile_pool(name="small", bufs=1))
        opool = stack.enter_context(tc.tile_pool(name="opool", bufs=1))
        psS = stack.enter_context(tc.tile_pool(name="psS", bufs=2, space="PSUM"))
        psAV = stack.enter_context(tc.tile_pool(name="psAV", bufs=2, space="PSUM"))

        # ---- persistent SBUF tiles (one tile per DMA chunk so compute
        # only waits for the chunks it actually reads) ----
        wq = [wpool.tile([128, FH], BF16, name=f"wq{i}") for i in range(NT)]
        wk = [wpool.tile([128, FH], BF16, name=f"wk{i}") for i in range(NT)]
        wv = [wpool.tile([128, FH], BF16, name=f"wv{i}") for i in range(NT)]
        wo = [wpool.tile([128, D], BF16, name=f"wo{i}") for i in range(NP)]
        qT = [inpool.tile([128, S], BF16, name=f"qT{i}") for i in range(NT)]
        kT = [inpool.tile([128, S], BF16, name=f"kT{i}") for i in range(NT)]
        vT = [inpool.tile([128, S], BF16, name=f"vT{i}") for i in range(NT)]
        mT = [inpool.tile([128, S], BF16, name=f"mT{i}") for i in range(NKT)]

        qs = [acts.tile([128, S], BF16, name=f"qs{p}") for p in range(NP)]
        ks = [acts.tile([128, S], BF16, name=f"ks{p}") for p in range(NP)]
        vsb = [acts.tile([128, NH, DK + 1], BF16, name=f"vsb{t}")
               for t in range(NKT)]
        attnT = [acts.tile([128, S], BF16, name=f"attnT{p}") for p in range(NP)]

        bq_sb = small.tile([128, NP], F32, name="bq_sb")
        bk_sb = small.tile([128, NP], F32, name="bk_sb")
        bo_sb = small.tile([128, NOT], F32, name="bo_sb")
        sel65 = small.tile([65, DK], BF16, name="sel65")
        denb = [small.tile([65, S], BF16, name=f"denb{h}") for h in range(2)]
        dummy = small.tile([1, 16], F32, name="dummy")

        nc.vector.memset(sel65[:], 0.0)
        nc.vector.memset(sel65[64:65, :], 1.0)
        for h in range(2):
            nc.vector.memset(denb[h][:], 0.0)
        nc.vector.memset(dummy[:], 0.0)
        # preload the exp activation table while DMAs stream
        nc.scalar.activation(dummy[:], dummy[:], Exp)
        for t in range(NKT):
            nc.vector.memset(vsb[t][:, :, DK:DK + 1], 1.0)

        # ---- chunked loads in priority order (all on the sync queue) ----
        nc.gpsimd.dma_start(bq_sb[:], dram["bqc"].ap())
        nc.gpsimd.dma_start(bk_sb[:], dram["bkc"].ap())
        nc.gpsimd.dma_start(bo_sb[:], dram["boc"].ap())
        qeng = [nc.sync, nc.scalar, nc.gpsimd]
        for dt in range(NT):
            qeng[dt % 3].dma_start(wv[dt][:], dram["wv"].ap()[dt])
            qeng[(dt + 1) % 3].dma_start(vT[dt][:], dram["vT"].ap()[dt])
        for dt in range(NT):
            nc.sync.dma_start(wq[dt][:], dram["wq"].ap()[dt])
        for dt in range(NT):
            nc.sync.dma_start(wk[dt][:], dram["wk"].ap()[dt])
        for dt in range(NT):
            nc.sync.dma_start(qT[dt][:], dram["qT"].ap()[dt])
        for dt in range(NT):
            nc.sync.dma_start(kT[dt][:], dram["kT"].ap()[dt])
        for kt in range(NKT):
            nc.sync.dma_start(mT[kt][:], dram["mT"].ap()[kt])
        for p in range(NP):
            nc.sync.dma_start(wo[p][:], dram["wo"].ap()[p])

        # PE warmup: dummy matmuls on memset tiles during the ~10us DMA
        # startup so the HAM clock-gate reaches 2.4GHz before real work.
        for i in range(16):
            wt = psS.tile([128, 2, FH], F32, tag="ps", name=f"warm{i}")
            nc.tensor.matmul(wt[0:64, 0, :], sel65[:], denb[i % 2][:, 0:512],
                             start=True, stop=True)

        # ---------------- V projection: vsb[tb] = (value @ Wv.T)[tb] -------
        # out [128 tokens, 512 features] per token block; dt-outer over the
        # first 4 blocks so compute starts after the first chunks land.
        def v_proj_group(tbs):
            tiles = {}
            for j, tb in enumerate(tbs):
                if j % 2 == 0:
                    t = psS.tile([128, 2, FH], F32, tag="ps", name=f"psv{tb}")
                tiles[tb] = (t, j % 2)
            for dt in range(NT):
                for tb in tbs:
                    t, half = tiles[tb]
                    nc.tensor.matmul(t[:, half, :],
                                     vT[dt][:, tb * 128:(tb + 1) * 128],
                                     wv[dt][:],
                                     start=(dt == 0), stop=(dt == NT - 1))
            for tb in tbs:
                t, half = tiles[tb]
                nc.vector.tensor_copy(
                    vsb[tb][:, :, 0:DK],
                    t[:, half, :].rearrange("p (h d) -> p h d", h=NH))

        v_proj_group([0, 1, 2, 3])

        # ---------------- Q/K projection for one pair ----------------------
        def q_proj(p):
            t = psS.tile([128, 2, FH], F32, tag="ps", name=f"psq{p}")
            for dt in range(NT):
                for qh in range(2):
                    nc.tensor.matmul(t[:, qh, :],
                                     wq[dt][:, p * 128:(p + 1) * 128],
                                     qT[dt][:, qh * 512:(qh + 1) * 512],
                                     start=(dt == 0), stop=(dt == NT - 1))
            nc.vector.tensor_scalar_add(qs[p][:],
                                        t[:].rearrange("p a b -> p (a b)"),
                                        bq_sb[:, p:p + 1])

        def k_proj(p):
            t = psS.tile([128, 2, FH], F32, tag="ps", name=f"psk{p}")
            for dt in range(NT):
                for qh in range(2):
                    nc.tensor.matmul(t[:, qh, :],
                                     wk[dt][:, p * 128:(p + 1) * 128],
                                     kT[dt][:, qh * 512:(qh + 1) * 512],
                                     start=(dt == 0), stop=(dt == NT - 1))
            nc.vector.tensor_scalar_add(ks[p][:],
                                        t[:].rearrange("p a b -> p (a b)"),
                                        bk_sb[:, p:p + 1])

        v_proj_group([4, 5, 6, 7])
        q_proj(0)
        k_proj(0)

        # ---------------- attention for one pair ---------------------------
        def attention_pair(p):
            avp = [psAV.tile([65, S], F32, tag="psAV", name=f"av{p}_{h}")
                   for h in range(2)]
            units = [(kt, qh) for kt in range(NKT) for qh in range(2)]
            exps = {}

            def av_mms(kt, qh):
                for h in range(2):
                    nc.tensor.matmul(avp[h][:, qh * 512:(qh + 1) * 512],
                                     vsb[kt][:, 2 * p + h, :],
                                     exps[(kt, qh)][:, h, :],
                                     start=(kt == 0), stop=(kt == NKT - 1))

            for u, (kt, qh) in enumerate(units):
                st = psS.tile([128, 2, 512], F32, tag="ps",
                              name=f"s{p}_{kt}_{qh}")
                for h in range(2):
                    nc.tensor.matmul(
                        st[:, h, :],
                        ks[p][h * 64:(h + 1) * 64, kt * 128:(kt + 1) * 128],
                        qs[p][h * 64:(h + 1) * 64, qh * 512:(qh + 1) * 512],
                        start=True, stop=True)
                ex = xp.tile([128, 2, 512], BF16, tag="ex", bufs=6,
                             name=f"ex{p}_{kt}_{qh}")
                nc.scalar.activation(ex[:], st[:], Exp)
                msl = mT[kt][:, qh * 512:(qh + 1) * 512]
                nc.vector.tensor_tensor(ex[:, 0, :], ex[:, 0, :], msl, Mult)
                nc.gpsimd.tensor_tensor(ex[:, 1, :], ex[:, 1, :], msl, Mult)
                exps[(kt, qh)] = ex
                if u >= DELAY:
                    av_mms(*units[u - DELAY])
                # HAM keep-warm filler: standalone LDWEIGHTS are pure PE
                # array activity (no PSUM write, no result).  Every real
                # matmul self-loads its weights, so these are inert.
                for j in range(5):
                    nc.tensor.ldweights(
                        ks[p][0:128, ((u + j) % 8) * 128:((u + j) % 8 + 1) * 128])
            for u in range(len(units) - DELAY, len(units)):
                av_mms(*units[u])
            # den rows -> zeroed bf16 staging (rows 0-63 stay zero), early
            # on the DVE queue so they complete during the next projections.
            for h in range(2):
                nc.vector.tensor_copy(denb[h][64:65, :], avp[h][64:65, :])
            return avp

        def norm_chain(p, avp):
            # selector matmul broadcasts the den row to partitions 0-63,
            # reciprocal on the broadcast, one multiply per head.  All the
            # waits are engine-local and short; no DMA mid-chain.
            for h in range(2):
                bcp = psS.tile([128, 2, 512], F32, tag="ps",
                               name=f"bcp{p}_{h}")
                for qh in range(2):
                    nc.tensor.matmul(bcp[0:64, qh, :], sel65[:],
                                     denb[h][:, qh * 512:(qh + 1) * 512],
                                     start=True, stop=True)
                bc = small.tile([64, S], F32, tag="bc", bufs=2,
                                name=f"bc{p}_{h}")
                nc.vector.reciprocal_approx_fast(
                    bc[:], bcp[0:64, :, :].rearrange("p a b -> p (a b)"))
                if h == 0:
                    nc.vector.tensor_tensor(attnT[p][0:64, :],
                                            avp[h][0:64, :], bc[:], Mult)
                else:
                    stg = small.tile([64, S], BF16, tag="stg", bufs=2,
                                     name=f"stg{p}")
                    nc.vector.tensor_tensor(stg[:], avp[h][0:64, :], bc[:],
                                            Mult)
                    nc.gpsimd.dma_start(attnT[p][64:128, :], stg[:])

        for p in range(NP):
            avp = attention_pair(p)
            norm_chain(p, avp)
            if p + 1 < NP:
                q_proj(p + 1)
                k_proj(p + 1)

        # ---------------- partial output projection ------------------------
        # ot processed in pairs; the dt=3 (last attention pair) contribution
        # is accumulated last so only it waits on the final normalize.
        for og in range(NOT // 2):
            ots = (2 * og, 2 * og + 1)
            tiles = {}
            for ot in ots:
                t = psS.tile([128, 2, 512], F32, tag="ps", name=f"pso{ot}")
                tiles[ot] = t
                for dt in range(NP - 1):
                    for qh in range(2):
                        nc.tensor.matmul(t[:, qh, :],
                                         wo[dt][:, ot * 128:(ot + 1) * 128],
                                         attnT[dt][:, qh * 512:(qh + 1) * 512],
                                         start=(dt == 0), stop=False)
            for ot in ots:
                t = tiles[ot]
                for qh in range(2):
                    nc.tensor.matmul(t[:, qh, :],
                                     wo[NP - 1][:, ot * 128:(ot + 1) * 128],
                                     attnT[NP - 1][:, qh * 512:(qh + 1) * 512],
                                     start=False, stop=True)
            for ot in ots:
                osb = opool.tile([128, S], BF16, tag="osb", bufs=2,
                                 name=f"osb{ot}")
                nc.scalar.activation(osb[:],
                                     tiles[ot][:].rearrange("p a b -> p (a b)"),
                                     Identity, bias=bo_sb[:, ot:ot + 1])
                nc.sync.dma_start(outT.ap()[ot], osb[:])


def _build():
    nc = bacc.Bacc("TRN2", target_bir_lowering=False, debug=False,
                   num_devices=NCORES)
    with tile.TileContext(nc) as tc:
        _emit(nc, tc)
    nc.compile()
    return nc


def _get_compiled():
    global _COMPILED
    if _COMPILED is None:
        _COMPILED = _build()
    return _COMPILED


def _bf(x):
    return np.ascontiguousarray(x).astype(ml_dtypes.bfloat16)


def kernel(**inputs):
    global LAST_RESULT
    query = np.asarray(inputs["query"], np.float32)
    key = np.asarray(inputs.get("key_in", inputs.get("key"))).astype(np.float32)
    value = np.asarray(inputs["value"], np.float32)
    mask = np.asarray(inputs["mask"])
    Wq = np.asarray(inputs["Wq"], np.float32)
    bq = np.asarray(inputs["bq"], np.float32)
    Wk = np.asarray(inputs["Wk"], np.float32)
    bk = np.asarray(inputs["bk"], np.float32)
    Wv = np.asarray(inputs["Wv"], np.float32)
    bv = np.asarray(inputs["bv"], np.float32)
    Wo = np.asarray(inputs["Wo"], np.float32)
    bo = np.asarray(inputs["bo"], np.float32)

    nc = _get_compiled()

    scale = np.float32(1.0 / np.sqrt(np.float32(DK)))
    WqT = Wq.T * scale          # [in, out]
    WkT = Wk.T
    WvT = Wv.T
    WoT = Wo.T                  # [in(=attn feature), out]
    bo_eff = bo + bv @ WoT      # fold V bias into the output bias

    per_hh = []
    for hh in range(2):
        fsl = slice(hh * FH, (hh + 1) * FH)
        m = {
            "wq": _bf(WqT[:, fsl].reshape(NT, 128, FH)),
            "wk": _bf(WkT[:, fsl].reshape(NT, 128, FH)),
            "wv": _bf(WvT[:, fsl].reshape(NT, 128, FH)),
            "wo": _bf(WoT[fsl, :].reshape(NP, 128, D)),
            "bqc": np.ascontiguousarray(
                (bq[fsl] * scale).reshape(NP, 128).T),
            "bkc": np.ascontiguousarray(bk[fsl].reshape(NP, 128).T),
            "boc": np.ascontiguousarray(bo_eff.reshape(NOT, 128).T)
            if hh == 0 else np.zeros((128, NOT), np.float32),
        }
        per_hh.append(m)

    per_b = []
    for b in range(B):
        m = {
            "qT": _bf(query[b].T.reshape(NT, 128, S)),
            "kT": _bf(key[b].T.reshape(NT, 128, S)),
            "vT": _bf(value[b].T.reshape(NT, 128, S)),
            "mT": _bf(mask[b, 0].T.astype(np.float32).reshape(NKT, 128, S)),
        }
        per_b.append(m)

    in_maps = []
    for c in range(NCORES):
        b, hh = divmod(c, 2)
        m = dict(per_hh[hh])
        m.update(per_b[b])
        in_maps.append(m)

    kwargs = {}
    if TRACE:
        kwargs = dict(trace=True, trace_cores=list(TRACE_CORES))
    res = bass_utils.run_bass_kernel_spmd(nc, in_maps,
                                          core_ids=list(range(NCORES)),
                                          **kwargs)
    LAST_RESULT = res

    out = np.empty((B, S, D), np.float32)
    for b in range(B):
        p0 = res.results[2 * b]["outT"].reshape(D, S).astype(np.float32)
        p1 = res.results[2 * b + 1]["outT"].reshape(D, S).astype(np.float32)
        out[b] = (p0 + p1).T
    return out
